# revision 1
# baseline (speedup 1.0000x reference)
"""TRN2 Bass kernel for nn_BlockPermProduct.

The reference applies 9 probabilistic block-permutation mixing steps to each
row of x [65536, 1024]. Every step is linear in x, so the whole transform is
``out = x @ M^T`` for a 1024x1024 matrix M that depends only on the tiny
(9, 3) logits. M^T is computed on the host in float64 by pushing the identity
matrix through the reference transform; the device kernel is then a dense
row-wise matmul:

  per 128-row tile:  xT = transpose(x_tile) on the PE (8 128x128 blocks),
                     out_tile = accumulate_{c} xT_c^T @ MT_c  into PSUM,
                     copy back to SBUF, DMA out.

Matmuls and transposes run in float32r (4-byte fp32 data with tf32-class
rounding in the PE): 1 cycle/row at N=512 vs 4 for plain fp32, measured
end-to-end rel err ~2.3e-4. Loads/stores are batched as 1 MiB transfers
(two row-tiles per DMA). The PE array trace shows ~0 idle between slices;
the kernel is PE-bound at ~1.7x the per-core HBM roofline.

Sharding: pure data parallel over the batch dim across 8 cores (SPMD, no
communication); M^T is replicated.
"""

import numpy as np
from contextlib import ExitStack

import concourse.bass as bass
import concourse.bacc as bacc
import concourse.mybir as mybir
import concourse.tile as tile
from concourse.bass_utils import run_bass_kernel_spmd

BATCH = 65536
SIZE = 1024
N_CORES = 8
ROWS_PER_CORE = BATCH // N_CORES  # 8192
P = 128
PAIR = 2  # row-tiles per DMA transfer (1 MiB)
N_STEPS = ROWS_PER_CORE // (P * PAIR)  # 32
N_CHUNK = SIZE // P  # 8
HALF = 512  # PSUM bank width in fp32

F32 = mybir.dt.float32
F32R = mybir.dt.float32r

# "f32"      : plain fp32 matmuls (safest numerics, 4 cyc/row)
# "f32r"     : f32r matmuls; fp32 DMA + fp32 PE transposes, rounding to f32r
#              at the PSUM->SBUF copy
# "f32r_dma" : f32r end-to-end including DMA dtype and f32r transposes
# "xbar"     : f32r matmuls; transposes via DMA XBAR on u16 hi/lo planes
#              (PE does matmuls only)
import os as _os
MATMUL_MODE = _os.environ.get("KMODE", "f32r_dma")

TRACE = False
TRACE_KWARGS = {}
LAST_RESULTS = None

_NC_CACHE = {}


def _transform64(y, logits):
    """Float64 port of the reference transform, applied to rows of y."""
    m = 10
    sizes = [SIZE >> i for i in range(m - 1)][::-1]  # [4, 8, ..., 1024]
    out = y
    for i in range(m - 2, -1, -1):
        n = sizes[i]
        p = 1.0 / (1.0 + np.exp(-logits[i].astype(np.float64)))
        z = out.reshape(-1, n)
        sep = z.reshape(-1, n // 2, 2).transpose(0, 2, 1).reshape(-1, n)
        z = (1 - p[0]) * z + p[0] * sep
        h = n // 2
        first = (1 - p[1]) * z[:, :h] + p[1] * z[:, h - 1::-1]
        second = (1 - p[2]) * z[:, h:] + p[2] * z[:, : h - 1 : -1]
        out = np.concatenate([first, second], axis=1).reshape(out.shape)
    return out


def _build_mt(logits):
    """M^T [1024, 1024] fp32: row j = transform(e_j), so MT[j, i] = M[i, j]."""
    eye = np.eye(SIZE, dtype=np.float64)
    mt = _transform64(eye, logits)
    return np.ascontiguousarray(mt.astype(np.float32))


def _build_bass(mode):
    xbar = mode == "xbar"
    f32r_dma = mode == "f32r_dma"
    mdt = F32 if mode == "f32" else F32R
    xdt = F32R if f32r_dma else F32  # dtype of x DMA + PE transposes
    U16 = mybir.dt.uint16
    nc = bacc.Bacc("TRN2", target_bir_lowering=False, debug=False)
    x = nc.dram_tensor("x", [ROWS_PER_CORE, SIZE], xdt, kind="ExternalInput").ap()
    mt = nc.dram_tensor("mt", [SIZE, SIZE], F32 if mode == "f32r" else mdt, kind="ExternalInput").ap()
    out = nc.dram_tensor(
        "out", [ROWS_PER_CORE, SIZE], F32, kind="ExternalOutput"
    ).ap()
    identd = nc.dram_tensor("ident", [P, P], xdt, kind="ExternalInput").ap()

    with tile.TileContext(nc) as tc, ExitStack() as ctx:
        const = ctx.enter_context(tc.tile_pool(name="const", bufs=1))
        if not xbar:
            # Identity arrives from the host (f32r-typed DMA producer) so the
            # first PE transposes don't wait on gpsimd/ACT preamble chains.
            ident = const.tile([P, P], xdt, tag="ident")
            nc.sync.dma_start(ident[:], identd[:])

        xpool = ctx.enter_context(tc.tile_pool(name="xin", bufs=4))

        # Kick off the first x load BEFORE the M^T loads so the PE's first
        # transposes aren't queued behind 4 MB of constants.
        xin0 = xpool.tile([P, PAIR * SIZE], xdt, tag="xin")
        nc.sync.dma_start(
            xin0[:].rearrange("p (s n) -> p s n", n=SIZE),
            x[0 : P * PAIR, :].rearrange("(s p) n -> p s n", p=P),
        )

        # M^T resident in SBUF as 8 per-chunk tiles; each matmul depends only
        # on its own chunk's DMA, so compute overlaps the constant loads.
        mts = []
        for c in range(N_CHUNK):
            t = const.tile([P, SIZE], F32 if mode == "f32r" else mdt, tag=f"mt{c}")
            nc.sync.dma_start(t[:], mt[c * P : (c + 1) * P, :])
            if mode == "f32r":
                tr = const.tile([P, SIZE], F32R, tag=f"mtr{c}")
                nc.vector.tensor_copy(tr[:], t[:])
                t = tr
            mts.append(t)
        xtpool = ctx.enter_context(tc.tile_pool(name="xtp", bufs=4))
        opool = ctx.enter_context(tc.tile_pool(name="osb", bufs=3))
        if xbar:
            planes = ctx.enter_context(tc.tile_pool(name="planes", bufs=3))
            pso = ctx.enter_context(tc.tile_pool(name="pso", bufs=4, space="PSUM"))
        else:
            pst = ctx.enter_context(tc.tile_pool(name="pst", bufs=2, space="PSUM"))
            pso = ctx.enter_context(tc.tile_pool(name="pso", bufs=2, space="PSUM"))

        for step in range(N_STEPS):
            r0 = step * P * PAIR
            if step == 0:
                xin = xin0
            else:
                # One 1 MiB load: PAIR row-tiles side by side in the free dim.
                xin = xpool.tile([P, PAIR * SIZE], xdt, tag="xin")
                nc.sync.dma_start(
                    xin[:].rearrange("p (s n) -> p s n", n=SIZE),
                    x[r0 : r0 + P * PAIR, :].rearrange("(s p) n -> p s n", p=P),
                )
            osb = opool.tile([P, PAIR * SIZE], F32, tag="osb")

            for s in range(PAIR):
                xv = xin[:, s * SIZE : (s + 1) * SIZE]
                if xbar:
                    # Deinterleave u16 hi/lo planes (compute engines allow
                    # strided APs), transpose each plane via the DMA XBAR,
                    # re-interleave, round to f32r. PE does matmuls only.
                    xv3 = xv.bitcast(U16).rearrange("p (k two) -> p k two", two=2)
                    lo_p = planes.tile([P, SIZE], U16, tag="lop")
                    hi_p = planes.tile([P, SIZE], U16, tag="hip")
                    nc.vector.tensor_copy(lo_p[:], xv3[:, :, 0])
                    nc.scalar.copy(hi_p[:], xv3[:, :, 1])
                    lo_t = planes.tile([P, SIZE], U16, tag="lot")
                    hi_t = planes.tile([P, SIZE], U16, tag="hit")
                    for c in range(N_CHUNK):
                        nc.sync.dma_start_transpose(
                            lo_t[:, c * P : (c + 1) * P],
                            lo_p[:, c * P : (c + 1) * P],
                        )
                        nc.scalar.dma_start_transpose(
                            hi_t[:, c * P : (c + 1) * P],
                            hi_p[:, c * P : (c + 1) * P],
                        )
                    xTm = xtpool.tile([P, SIZE], F32, tag="xtm")
                    m3 = xTm[:].bitcast(U16).rearrange("p (k two) -> p k two", two=2)
                    nc.vector.tensor_copy(m3[:, :, 0], lo_t[:])
                    nc.scalar.copy(m3[:, :, 1], hi_t[:])
                    xT = xtpool.tile([P, SIZE], mdt, tag="xt")
                    nc.scalar.copy(xT[:], xTm[:])  # rounding producer for f32r
                else:
                    # Transpose the 8 [128,128] blocks on the PE; copy to SBUF.
                    xT = xtpool.tile([P, SIZE], mdt, tag="xt")
                    for half in range(2):
                        tp = pst.tile([P, HALF], xdt, tag=f"tp{half}")
                        for q in range(4):
                            c = half * 4 + q
                            nc.tensor.transpose(
                                tp[:, q * P : (q + 1) * P],
                                xv[:, c * P : (c + 1) * P],
                                ident[:],
                            )
                        nc.scalar.copy(xT[:, half * HALF : (half + 1) * HALF], tp[:])

                # out_tile[r, i] = sum_c xT_c^T @ MT_c ; two PSUM banks.
                for h in range(2):
                    po = pso.tile([P, HALF], F32, tag=f"po{h}")
                    for c in range(N_CHUNK):
                        nc.tensor.matmul(
                            po[:],
                            xT[:, c * P : (c + 1) * P],
                            mts[c][:, h * HALF : h * HALF + HALF],
                            start=(c == 0),
                            stop=(c == N_CHUNK - 1),
                        )
                    nc.vector.tensor_copy(
                        osb[:, s * SIZE + h * HALF : s * SIZE + (h + 1) * HALF],
                        po[:],
                    )

            nc.sync.dma_start(
                out[r0 : r0 + P * PAIR, :].rearrange("(s p) n -> p s n", p=P),
                osb[:].rearrange("p (s n) -> p s n", n=SIZE),
            )

    nc.compile()
    return nc


def _get_nc():
    key = MATMUL_MODE
    if key not in _NC_CACHE:
        _NC_CACHE[key] = _build_bass(key)
    return _NC_CACHE[key]


def kernel(x, logits):
    x = np.ascontiguousarray(np.asarray(x), dtype=np.float32)
    logits = np.asarray(logits)
    assert x.shape == (BATCH, SIZE)

    mt = _build_mt(logits)
    nc = _get_nc()

    ident = np.eye(P, dtype=np.float32)
    in_maps = [
        {
            "x": x[i * ROWS_PER_CORE : (i + 1) * ROWS_PER_CORE],
            "mt": mt,
            "ident": ident,
        }
        for i in range(N_CORES)
    ]
    kwargs = dict(TRACE_KWARGS)
    if TRACE:
        kwargs.setdefault("trace", True)
        kwargs.setdefault("trace_cores", [0])
    res = run_bass_kernel_spmd(nc, in_maps, core_ids=list(range(N_CORES)), **kwargs)
    global LAST_RESULTS
    LAST_RESULTS = res
    return np.concatenate([res.results[i]["out"] for i in range(N_CORES)], axis=0)



# revision 3
# speedup vs baseline: 1.2549x; 1.2549x over previous
"""TRN2 Bass kernel for nn_BlockPermProduct.

The reference applies 9 probabilistic block-permutation mixing steps to each
row of x [65536, 1024]; the whole transform is linear: out = M x per row,
with M depending only on the tiny (9, 3) logits. Instead of the dense
1024x1024 matmul (PE-bound, ~313 us), this kernel exploits the structure:

  M = D512 . Sh1024,  Sh1024 = (1-p0) (I + beta S),  beta = p0/(1-p0)

where S is the parity sort of the 1024 columns (the "even_odd" shuffle at
block size 1024) and D512 = B4...B512 . Rv1024 is block-diagonal with two
512x512 blocks -- all smaller-block steps nest inside the 512 boundaries.
This halves the matmul MACs.

Device layout is TRANSPOSED (x^T tiles: partition = column chunk, free =
rows), so no PE transposes are needed anywhere:

  per 512-row super-tile (16 per core):
    1. DMA in 8 chunk tiles y = x^T[:, rows]            (1 MiB fp16)
    2. S gather: 4 partition-strided SBUF->SBUF DMAs build sx = (Sx)^T
    3. one DVE scalar_tensor_tensor: u = beta*sx + y     (fp16, 2x mode)
    4. 32 matmuls (N=512, fp16, stationaries SBUF-resident):
         out^T[i] = sum_c AhatT[c-block]^T @ u[c]   into 8 PSUM banks
    5. PSUM->SBUF fp16 casts split between ACT and DVE
    6. DMA out 1 MiB fp16

Everything on device is fp16 (abs-max rel err ~2e-4 vs the 2e-2 budget);
the host does the f32<->f16 transpose/cast and builds the ~0.5 MiB of
matrix constants from the logits in float64.

Sharding: pure data parallel over the batch dim across 8 cores (SPMD,
no communication); the constants are replicated.
"""

import numpy as np
from contextlib import ExitStack

import concourse.bass as bass
import concourse.bacc as bacc
import concourse.mybir as mybir
import concourse.tile as tile
from concourse.bass_utils import run_bass_kernel_spmd

BATCH = 65536
SIZE = 1024
N_CORES = 8
ROWS_PER_CORE = BATCH // N_CORES  # 8192
P = 128
NROW = 512                        # rows per super-tile
N_ST = ROWS_PER_CORE // NROW      # 16
N_CHUNK = SIZE // P               # 8

F16 = mybir.dt.float16
F32 = mybir.dt.float32

MATMUL_MODE = "fp16_blockdiag512"

TRACE = False
TRACE_KWARGS = {}
LAST_RESULTS = None

_NC_CACHE = {}


def _transform64(y, logits, skip_sh1024=False):
    """Float64 port of the reference transform, applied to rows of y."""
    m = 10
    sizes = [SIZE >> i for i in range(m - 1)][::-1]  # [4, 8, ..., 1024]
    out = y
    for i in range(m - 2, -1, -1):
        n = sizes[i]
        p = 1.0 / (1.0 + np.exp(-logits[i].astype(np.float64)))
        z = out.reshape(-1, n)
        if not (skip_sh1024 and i == m - 2):
            sep = z.reshape(-1, n // 2, 2).transpose(0, 2, 1).reshape(-1, n)
            z = (1 - p[0]) * z + p[0] * sep
        h = n // 2
        first = (1 - p[1]) * z[:, :h] + p[1] * z[:, h - 1::-1]
        second = (1 - p[2]) * z[:, h:] + p[2] * z[:, : h - 1 : -1]
        out = np.concatenate([first, second], axis=1).reshape(out.shape)
    return out


def _build_constants(logits):
    """beta and the stationary blocks atT [1024, 512] from the logits.

    D512 = B4...B512 . Rv1024 is block-diagonal (two 512 blocks); with
    Ahat = (1-p0) D512 the device computes out = Ahat (x + beta Sx).
    atT stacks the two diagonal blocks of Ahat^T = (1-p0) D512T.
    """
    l64 = np.asarray(logits, dtype=np.float64)
    p0 = 1.0 / (1.0 + np.exp(-l64[8, 0]))
    beta = p0 / (1.0 - p0)
    d512t = _transform64(np.eye(SIZE), l64, skip_sh1024=True)
    assert abs(d512t[:512, 512:]).max() == 0.0
    assert abs(d512t[512:, :512]).max() == 0.0
    ahat_t = (1.0 - p0) * d512t
    at = np.concatenate([ahat_t[:512, :512], ahat_t[512:, 512:]], axis=0)
    return float(beta), np.ascontiguousarray(at.astype(np.float16))


def _build_bass():
    nc = bacc.Bacc("TRN2", target_bir_lowering=False, debug=False)
    xt = nc.dram_tensor("xt", [SIZE, ROWS_PER_CORE], F16, kind="ExternalInput").ap()
    at = nc.dram_tensor("at", [SIZE, 512], F16, kind="ExternalInput").ap()
    beta = nc.dram_tensor("beta", [P, 1], F32, kind="ExternalInput").ap()
    outt = nc.dram_tensor(
        "outt", [SIZE, ROWS_PER_CORE], F16, kind="ExternalOutput"
    ).ap()

    mult = mybir.AluOpType.mult
    add = mybir.AluOpType.add

    with tile.TileContext(nc) as tc, ExitStack() as ctx:
        const = ctx.enter_context(tc.tile_pool(name="const", bufs=1))
        ypool = ctx.enter_context(tc.tile_pool(name="yin", bufs=3))
        sxpool = ctx.enter_context(tc.tile_pool(name="sx", bufs=2))
        upool = ctx.enter_context(tc.tile_pool(name="u", bufs=2))
        opool = ctx.enter_context(tc.tile_pool(name="osb", bufs=2))
        pspool = ctx.enter_context(tc.tile_pool(name="ps", bufs=1, space="PSUM"))

        # First x load before the constants so the front of the pipeline
        # isn't queued behind them.
        y0 = ypool.tile([P, N_CHUNK * NROW], F16, tag="y")
        nc.sync.dma_start(
            y0[:].rearrange("p (c n) -> p c n", n=NROW),
            xt[:, 0:NROW].rearrange("(c p) n -> p c n", p=P),
        )

        bt = const.tile([P, 1], F32, tag="beta")
        nc.sync.dma_start(bt[:], beta[:])
        ats = []
        for c in range(N_CHUNK):
            t = const.tile([P, 512], F16, tag=f"at{c}")
            nc.sync.dma_start(t[:], at[c * P : (c + 1) * P, :])
            ats.append(t)

        for s in range(N_ST):
            rs = s * NROW
            if s == 0:
                y = y0
            else:
                y = ypool.tile([P, N_CHUNK * NROW], F16, tag="y")
                nc.sync.dma_start(
                    y[:].rearrange("p (c n) -> p c n", n=NROW),
                    xt[:, rs : rs + NROW].rearrange("(c p) n -> p c n", p=P),
                )
            yv = y[:].rearrange("p (c n) -> p c n", n=NROW)

            # sx = (Sx)^T chunks: S is the parity sort, so chunk c of Sx is
            # an interleave of two contiguous 64-partition runs of y.
            sx = sxpool.tile([P, N_CHUNK * NROW], F16, tag="sx")
            sxv = sx[:].rearrange("p (c n) -> p c n", n=NROW)
            nc.sync.dma_start(sxv[0:64, 0:4, :], yv[0:128:2, 0:8:2, :])
            nc.sync.dma_start(sxv[64:128, 0:4, :], yv[0:128:2, 1:8:2, :])
            nc.sync.dma_start(sxv[0:64, 4:8, :], yv[1:128:2, 0:8:2, :])
            nc.sync.dma_start(sxv[64:128, 4:8, :], yv[1:128:2, 1:8:2, :])

            # u = beta * sx + y in one DVE pass (fp16 2x mode)
            u = upool.tile([P, N_CHUNK * NROW], F16, tag="u")
            nc.vector.scalar_tensor_tensor(
                u[:], sx[:], bt[:, 0:1], y[:], op0=mult, op1=add
            )

            osb = opool.tile([P, N_CHUNK * NROW], F16, tag="osb")
            for i in range(N_CHUNK):
                b = i // 4
                ps = pspool.tile([P, NROW], F32, tag=f"ps{i}")
                for c0 in range(4):
                    cg = 4 * b + c0
                    nc.tensor.matmul(
                        ps[:],
                        ats[cg][:, 128 * (i % 4) : 128 * (i % 4) + 128],
                        u[:, cg * NROW : (cg + 1) * NROW],
                        start=(c0 == 0),
                        stop=(c0 == 3),
                    )
                dst = osb[:, i * NROW : (i + 1) * NROW]
                if i % 2 == 0:
                    nc.scalar.copy(dst, ps[:])
                else:
                    nc.vector.tensor_copy(dst, ps[:])

            nc.sync.dma_start(
                outt[:, rs : rs + NROW].rearrange("(c p) n -> p c n", p=P),
                osb[:].rearrange("p (c n) -> p c n", n=NROW),
            )

    nc.compile()
    return nc


def _get_nc():
    key = MATMUL_MODE
    if key not in _NC_CACHE:
        _NC_CACHE[key] = _build_bass()
    return _NC_CACHE[key]


def kernel(x, logits):
    x = np.asarray(x)
    logits = np.asarray(logits)
    assert x.shape == (BATCH, SIZE)

    beta, at = _build_constants(logits)
    assert beta < 60000.0, f"beta={beta} would overflow fp16 intermediates"
    beta_arr = np.full((P, 1), beta, dtype=np.float32)

    nc = _get_nc()

    in_maps = []
    for i in range(N_CORES):
        xc = x[i * ROWS_PER_CORE : (i + 1) * ROWS_PER_CORE]
        in_maps.append(
            {
                "xt": np.ascontiguousarray(xc.T.astype(np.float16, copy=False)),
                "at": at,
                "beta": beta_arr,
            }
        )
    kwargs = dict(TRACE_KWARGS)
    if TRACE:
        kwargs.setdefault("trace", True)
        kwargs.setdefault("trace_cores", [0])
    res = run_bass_kernel_spmd(nc, in_maps, core_ids=list(range(N_CORES)), **kwargs)
    global LAST_RESULTS
    LAST_RESULTS = res

    out = np.empty((BATCH, SIZE), dtype=np.float32)
    for i in range(N_CORES):
        out[i * ROWS_PER_CORE : (i + 1) * ROWS_PER_CORE] = (
            res.results[i]["outt"].T.astype(np.float32)
        )
    return out
